# revision 11
# baseline (speedup 1.0000x reference)
"""Distributed causal-attention-with-dropout kernel for 8 TRN2 NeuronCores, v17.

Architecture ("all-local projections", fully static SPMD graph):

- Host pre-formats inputs (layout only, all model FLOPs stay on device):
  each core receives xq = x^T columns of its 4 OWNED q-tiles
  {c, 15-c, 16+c, 31-c} (bf16), the FULL Wq^T / Wk^T / Wv^T (bf16,
  replicated), dropout-mask rows of its owned tiles (bf16), and the causal
  threshold table.  There is NO x gather at all.
- A tiny dummy AllGather with NO input dependency fires at t~0 so the
  collectives-runtime init barrier (~20+60us) overlaps the K projection.
- Tensor phase order: K proj -> V slots {0,1} -> V slots {2,3} -> Q proj
  -> attention.  Startup loads are interleaved (wk ki-chunk, xq ki-chunk)
  across sync+gpsimd queues and the K projection consumes ki in arrival
  order, so the PE starts at ~4us and stays dense (HAM stays un-throttled).
- K^T is AllGathered in k-tile-group chunks at FULL d_out depth:
  kq_in[g] = [2048, 256] = all d_out rows x the core's two owned q-tiles
  of group g (g=0: tiles {c,15-c} < 16; g=1: tiles {16+c,31-c}).  Score
  big-block B therefore needs ONLY kg[B//2].
- CC chain order = consumption order: KAG0, VAG0, KAG1, VAG1.  The
  KAG1/VAG1 triggers are EMITTED after the B=0/1 attention loads: DRAM
  DMAs conservatively wait on previously-emitted collectives, so B=0/1
  loads must precede the later triggers or they stall until KAG1 is done
  (measured 80us loss in v16).
- Attention: core c owns q-tiles {c, 15-c, 16+c, 31-c}; k-blocks are 1024
  wide, giving a ZERO-padding static schedule (slot s needs exactly s+1
  blocks; 10 pairs).  Causality enforced per-row by (iota(p-j) >= thr) * P
  on the vector engine; softmax without max-subtraction; denominators use
  pre-dropout sums.  Pair p's P-transposes and attn@V run after pair p+1's
  score matmuls (software pipeline).
"""

import math
import os
import sys
from contextlib import ExitStack

import numpy as np
import ml_dtypes

for _p in ("/opt/trn_rl_repo", "/root/.axon_site/_ro/trn_rl_repo"):
    if os.path.isdir(_p) and _p not in sys.path:
        sys.path.append(_p)

import concourse.bass as bass
import concourse.tile as tile
from concourse import bacc, mybir
from concourse import bass_utils
from concourse.masks import make_identity

S, D = 4096, 2048
NC = 8
SB = 512          # seq rows per core (4 owned 128-tiles)
BK = 1024         # big k-block width
NBIG = 4
KBMAX = (1, 2, 3, 4)
PBASE = (0, 1, 3, 6)
NPAIR = 10
SCALE = 1.0 / math.sqrt(float(D))
F32 = mybir.dt.float32
BF16 = mybir.dt.bfloat16
RG = [list(range(NC))]
ALU = mybir.AluOpType
AFT = mybir.ActivationFunctionType

# ki consumption order for the K projection: matches the 2-queue load
# arrival pattern (sync: wkA+xq ki 0..7, gpsimd: wkB+xq ki 8..15) so the
# first dp group issues matmuls as chunks land.
KI_ORDER = (0, 8, 1, 9, 2, 10, 3, 11, 4, 12, 5, 13, 6, 14, 7, 15)


def owned_tiles(c):
    return (c, 15 - c, 16 + c, 31 - c)


def tile_owner_slot(t):
    if t <= 7:
        return t, 0
    if t <= 15:
        return 15 - t, 1
    if t <= 23:
        return t - 16, 2
    return 31 - t, 3


# row of tile t inside its V AllGather chunk (chunk = t//16; within a
# chunk, rank blocks of 256 rows hold slots {0,1} or {2,3})
VROW2 = [256 * tile_owner_slot(t)[0] + 128 * (tile_owner_slot(t)[1] % 2)
         for t in range(32)]


def build():
    nc = bacc.Bacc("TRN2", target_bir_lowering=False, debug=False,
                   num_devices=NC)

    xq_in = nc.dram_tensor("xq", [D, SB], BF16, kind="ExternalInput").ap()
    wq_in = nc.dram_tensor("wqT", [D, D], BF16, kind="ExternalInput").ap()
    wv_in = nc.dram_tensor("wvT", [D, D], BF16, kind="ExternalInput").ap()
    wk_in = nc.dram_tensor("wkT", [D, D], BF16, kind="ExternalInput").ap()
    mask_in = nc.dram_tensor("drop_mask", [4 * 128, S], BF16,
                             kind="ExternalInput").ap()
    sched_in = nc.dram_tensor("sched", [128, NPAIR], F32,
                              kind="ExternalInput").ap()
    out_ext = nc.dram_tensor("out", [4 * 128, D], BF16,
                             kind="ExternalOutput").ap()

    with tile.TileContext(nc) as tc:
        with ExitStack() as es:
            dram = es.enter_context(tc.tile_pool(name="dram", bufs=1,
                                                 space="DRAM"))
            const = es.enter_context(tc.tile_pool(name="const", bufs=1))
            psum = es.enter_context(tc.tile_pool(name="psum", bufs=1,
                                                 space="PSUM"))

            # ---------------- DRAM scratch ----------------
            dummy_in = dram.tile([1, NPAIR], F32, name="dummy_in")
            dummy_out = dram.tile([NC, NPAIR], F32, addr_space="Shared",
                                  name="dummy_out")
            # V contributions split by slot-pair: chunk 0 = slots {0,1}
            # (true tiles 0..15), chunk 1 = slots {2,3} (tiles 16..31).
            vq_in = [dram.tile([256, D], BF16, name=f"vq_in{h}")
                     for h in range(2)]
            vg = [dram.tile([NC * 256, D], BF16, addr_space="Shared",
                            name=f"vg{h}") for h in range(2)]
            # per-core K^T contribution in k-tile-group chunks at FULL
            # d_out depth: chunk g = [2048 d_out rows, 256 cols] covering
            # the core's two owned q-tiles of group g.
            kq_in = [dram.tile([D, 256], BF16, name=f"kq_in{g}")
                     for g in range(2)]
            kg = [dram.tile([NC * D, 256], BF16, addr_space="Shared",
                            name=f"kg{g}") for g in range(2)]

            # dummy AllGather first, with NO input dependency (dummy_in is
            # never written): the CC trigger fires at t~0 so the
            # collectives-init barrier overlaps the K projection.
            nc.gpsimd.collective_compute(
                "AllGather", ALU.bypass, replica_groups=RG,
                ins=[dummy_in.opt()], outs=[dummy_out.opt()],
            )

            # ---------------- weight / activation loads ----------------
            sched_sb = const.tile([128, NPAIR], F32, name="sched_sb")
            nc.sync.dma_start(sched_sb[:], sched_in)

            att = es.enter_context(tc.tile_pool(name="att", bufs=1))
            qt_sb = att.tile([128, 16, SB], BF16, name="qt_sb")

            qes = ExitStack()
            xqp = qes.enter_context(tc.tile_pool(name="xqp", bufs=1))
            xq_sb = xqp.tile([128, 16, SB], BF16, name="xq_sb")
            stagep = qes.enter_context(tc.tile_pool(name="stagep", bufs=2))
            # Weight pool: 4 buffers of [128, 8, D] (one ki-half each).
            # Rotation: wkA, wkB, wvA, wvB live first; wqA/wqB rotate into
            # wkA/wkB's buffers once the K projection finishes.
            wpool = qes.enter_context(tc.tile_pool(name="wpool", bufs=4))

            def wh_tile(name):
                return wpool.tile([128, 8, D], BF16, tag="wh", name=name)

            wkA = wh_tile("wkA"); wkB = wh_tile("wkB")
            wvA = wh_tile("wvA"); wvB = wh_tile("wvB")

            # fine-grained interleaved startup loads:
            # sync:   (wkA ki, xq ki) for ki 0..7
            # gpsimd: (wkB ki, xq ki) for ki 8..15
            # scalar: wvA, wvB (then wq, masks later)
            for ki in range(8):
                nc.sync.dma_start(wkA[:, ki, :],
                                  wk_in[128 * ki:128 * (ki + 1), :])
                nc.sync.dma_start(xq_sb[:, ki, :],
                                  xq_in[128 * ki:128 * (ki + 1), :])
            for ki in range(8, 16):
                nc.gpsimd.dma_start(wkB[:, ki - 8, :],
                                    wk_in[128 * ki:128 * (ki + 1), :])
                nc.gpsimd.dma_start(xq_sb[:, ki, :],
                                    xq_in[128 * ki:128 * (ki + 1), :])
            nc.scalar.dma_start(
                wvA[:], wv_in[0:1024, :].rearrange("(k p) d -> p k d", p=128))
            nc.scalar.dma_start(
                wvB[:], wv_in[1024:2048, :].rearrange("(k p) d -> p k d",
                                                      p=128))

            # ---------------- constants (gpsimd engine, after triggers) ---
            ident_sb = const.tile([128, 128], BF16, name="ident_sb")
            make_identity(nc, ident_sb[:])
            iota_sb = const.tile([128, BK], F32, name="iota_sb")
            nc.gpsimd.iota(
                iota_sb[:], pattern=[[-1, BK]], base=0, channel_multiplier=1,
                allow_small_or_imprecise_dtypes=True,
            )

            partials = const.tile([128, NPAIR], F32, name="partials")
            den = const.tile([128, 4], F32, name="den")
            rec = const.tile([128, 4], F32, name="rec")

            # ------- phase K: local K^T (all d_out, own q), k-tile-group
            # chunked output at full depth -------
            for dp in range(8):
                psA = psum.tile([128, BK], F32, tag="pw", bufs=3,
                                name=f"pskA{dp}")
                psB = psum.tile([128, BK], F32, tag="pw", bufs=3,
                                name=f"pskB{dp}")
                for i, ki in enumerate(KI_ORDER):
                    wkh = wkA if ki < 8 else wkB
                    nc.tensor.matmul(
                        psA[:, 0:SB], lhsT=wkh[:, ki % 8, 256 * dp:
                                               256 * dp + 128],
                        rhs=xq_sb[:, ki, :],
                        start=(i == 0), stop=(i == 15),
                    )
                    nc.tensor.matmul(
                        psB[:, 0:SB], lhsT=wkh[:, ki % 8, 256 * dp + 128:
                                               256 * dp + 256],
                        rhs=xq_sb[:, ki, :],
                        start=(i == 0), stop=(i == 15),
                    )
                for half, ps in ((0, psA), (1, psB)):
                    dt = 2 * dp + half
                    kst = stagep.tile([128, SB], BF16, tag="kst", bufs=4,
                                      name=f"kst{dt}")
                    nc.vector.tensor_copy(kst[:], ps[:, 0:SB])
                    # group 0 = slots {0,1} = cols 0:256; group 1 = 256:512
                    nc.sync.dma_start(
                        kq_in[0][128 * dt:128 * (dt + 1), :], kst[:, 0:256])
                    nc.sync.dma_start(
                        kq_in[1][128 * dt:128 * (dt + 1), :], kst[:, 256:512])

            nc.gpsimd.collective_compute(
                "AllGather", ALU.bypass, replica_groups=RG,
                ins=[kq_in[0].opt()], outs=[kg[0].opt()],
            )

            # ------- phase V: natural layout; slots {0,1} then {2,3} ------
            def v_slot(st):
                vq_dst = vq_in[st // 2].rearrange("(t p) d -> p t d", p=128)
                for h in range(2):
                    ps = psum.tile([128, BK], F32, tag="pw", bufs=3,
                                   name=f"psv{st}_{h}")
                    for ki in range(16):
                        for n2 in range(2):
                            nc.tensor.matmul(
                                ps[:, 512 * n2:512 * (n2 + 1)],
                                lhsT=xq_sb[:, ki, 128 * st:128 * (st + 1)],
                                rhs=(wvA if ki < 8 else wvB)[
                                    :, ki % 8, BK * h + 512 * n2:
                                    BK * h + 512 * (n2 + 1)],
                                start=(ki == 0), stop=(ki == 15),
                                skip_group_check=True,
                            )
                    vst = stagep.tile([128, BK], BF16, tag="vst", bufs=3,
                                      name=f"vst{st}_{h}")
                    nc.vector.tensor_copy(vst[:], ps[:])
                    nc.sync.dma_start(
                        vq_dst[:, st % 2, BK * h:BK * (h + 1)], vst[:])

            v_slot(0)
            v_slot(1)
            nc.gpsimd.collective_compute(
                "AllGather", ALU.bypass, replica_groups=RG,
                ins=[vq_in[0].opt()], outs=[vg[0].opt()],
            )
            v_slot(2)
            v_slot(3)

            # Wq^T halves rotate into Wk^T's buffers (dep: K matmuls done).
            wqA = wh_tile("wqA"); wqB = wh_tile("wqB")
            nc.scalar.dma_start(
                wqA[:], wq_in[0:1024, :].rearrange("(k p) d -> p k d", p=128))
            nc.scalar.dma_start(
                wqB[:], wq_in[1024:2048, :].rearrange("(k p) d -> p k d",
                                                      p=128))

            # ------- phase Q: local Q^T projection ------
            for dp in range(8):
                psA = psum.tile([128, BK], F32, tag="pw", bufs=3,
                                name=f"psqA{dp}")
                psB = psum.tile([128, BK], F32, tag="pw", bufs=3,
                                name=f"psqB{dp}")
                for ki in range(16):
                    wqh = wqA if ki < 8 else wqB
                    nc.tensor.matmul(
                        psA[:, 0:SB], lhsT=wqh[:, ki % 8, 256 * dp:
                                               256 * dp + 128],
                        rhs=xq_sb[:, ki, :],
                        start=(ki == 0), stop=(ki == 15),
                    )
                    nc.tensor.matmul(
                        psB[:, 0:SB], lhsT=wqh[:, ki % 8, 256 * dp + 128:
                                               256 * dp + 256],
                        rhs=xq_sb[:, ki, :],
                        start=(ki == 0), stop=(ki == 15),
                    )
                nc.vector.tensor_copy(qt_sb[:, 2 * dp, :], psA[:, 0:SB])
                nc.vector.tensor_copy(qt_sb[:, 2 * dp + 1, :],
                                      psB[:, 0:SB])
            qes.close()

            # ---------------- attention (software-pipelined) ----------------
            accp = es.enter_context(tc.tile_pool(name="accp", bufs=1))
            ktl = es.enter_context(tc.tile_pool(name="ktl", bufs=3))
            vtl = es.enter_context(tc.tile_pool(name="vtl", bufs=3))
            mkl = es.enter_context(tc.tile_pool(name="mkl", bufs=4))
            pwork = es.enter_context(tc.tile_pool(name="pwork", bufs=2))

            acc = [accp.tile([128, D], F32, name=f"acc{t}") for t in range(4)]

            def normalize_slot(slot):
                obf = pwork.tile([128, D], BF16, tag="obf", bufs=2,
                                 name=f"obf{slot}")
                nc.vector.tensor_reduce(
                    den[:, slot:slot + 1],
                    partials[:, PBASE[slot]:PBASE[slot] + KBMAX[slot]],
                    axis=mybir.AxisListType.X, op=ALU.add,
                )
                nc.vector.reciprocal(rec[:, slot:slot + 1],
                                     den[:, slot:slot + 1])
                nc.vector.tensor_scalar_mul(
                    obf[:], acc[slot][:], rec[:, slot:slot + 1])
                nc.scalar.dma_start(
                    out_ext[128 * slot:128 * (slot + 1), :], obf[:])

            def tp_stage(st):
                pm, vtA, vtB, B, slot = st
                pmt = pwork.tile([128, 8, 128], BF16, tag="pmt",
                                 name=f"pmt{B}_{slot}")
                for j in range(8):
                    tp = psum.tile([128, 128], BF16, tag="tp", bufs=2,
                                   name=f"tp{B}_{slot}_{j}")
                    nc.tensor.matmul(
                        tp[:], lhsT=pm[:, 128 * j:128 * (j + 1)],
                        rhs=ident_sb[:], is_transpose=True,
                        skip_group_check=True)
                    nc.scalar.copy(pmt[:, j, :], tp[:])
                return pmt

            def av_stage(st, pmt):
                pm, vtA, vtB, B, slot = st
                for h, vt in ((0, vtA), (1, vtB)):
                    av = psum.tile([128, BK], F32, tag="pw", bufs=3,
                                   name=f"av{B}_{slot}_{h}")
                    for j in range(8):
                        for n2 in range(2):
                            nc.tensor.matmul(
                                av[:, 512 * n2:512 * (n2 + 1)],
                                lhsT=pmt[:, j, :],
                                rhs=vt[:, j, 512 * n2:512 * (n2 + 1)],
                                start=(j == 0), stop=(j == 7),
                                skip_group_check=True,
                            )
                    if B == 0:
                        nc.vector.tensor_copy(
                            acc[slot][:, BK * h:BK * (h + 1)], av[:])
                    else:
                        nc.vector.scalar_tensor_tensor(
                            out=acc[slot][:, BK * h:BK * (h + 1)],
                            in0=av[:], scalar=1.0,
                            in1=acc[slot][:, BK * h:BK * (h + 1)],
                            op0=ALU.mult, op1=ALU.add,
                        )

            state = {"prev": None, "prev_pmt": None}

            def emit_block(B):
                # kt halves: ktA = d_out rows 0..1023 (score ki 0..7),
                # ktB = rows 1024..2047 (ki 8..15), both from kg[B//2].
                # All 8 tiles of big-block B share the same slot parity:
                # columns 128*(B%2) of the AG chunk.
                ktA = ktl.tile([128, 8, BK], BF16, tag="kt", name=f"ktA{B}")
                ktB = ktl.tile([128, 8, BK], BF16, tag="kt", name=f"ktB{B}")
                g = B // 2
                coff = 128 * (B % 2)
                for H, kth, eng in ((0, ktA, nc.sync), (1, ktB, nc.gpsimd)):
                    for j in range(8):
                        t = 8 * B + j
                        c, _s = tile_owner_slot(t)
                        eng.dma_start(
                            kth[:, :, 128 * j:128 * (j + 1)],
                            kg[g][D * c + BK * H:D * c + BK * (H + 1),
                                  coff:coff + 128]
                            .rearrange("(k p) q -> p k q", p=128),
                        )
                vtA = vtl.tile([128, 8, BK], BF16, tag="vt", name=f"vtA{B}")
                vtB = vtl.tile([128, 8, BK], BF16, tag="vt", name=f"vtB{B}")
                for j in range(8):
                    t = 8 * B + j
                    vgx = vg[t // 16]
                    r0 = VROW2[t]
                    nc.sync.dma_start(vtA[:, j, :], vgx[r0:r0 + 128, 0:BK])
                    nc.scalar.dma_start(vtB[:, j, :], vgx[r0:r0 + 128, BK:D])
                for slot in range(B, 4):
                    p = PBASE[slot] + B
                    mk = mkl.tile([128, BK], BF16, tag="mk",
                                  name=f"mk{B}_{slot}")
                    nc.scalar.dma_start(
                        mk[:],
                        mask_in[128 * slot:128 * (slot + 1),
                                BK * B:BK * (B + 1)],
                    )
                    sc = psum.tile([128, BK], F32, tag="pw", bufs=3,
                                   name=f"sc{B}_{slot}")
                    for ki in range(16):
                        if ki == 8 and state["prev"] is not None:
                            # interleave prev pair's P-transposes here so
                            # the pmt copies finish before its attn@V
                            state["prev_pmt"] = tp_stage(state["prev"])
                        kth = ktA if ki < 8 else ktB
                        for n2 in range(2):
                            nc.tensor.matmul(
                                sc[:, 512 * n2:512 * (n2 + 1)],
                                lhsT=qt_sb[:, ki, 128 * slot:128 * (slot + 1)],
                                rhs=kth[:, ki % 8, 512 * n2:512 * (n2 + 1)],
                                start=(ki == 0), stop=(ki == 15),
                                skip_group_check=True,
                            )
                    pex = pwork.tile([128, BK], BF16, tag="pex", bufs=1,
                                     name=f"pex{B}_{slot}")
                    nc.scalar.activation(pex[:], sc[:], AFT.Exp, scale=SCALE)
                    pcs = pwork.tile([128, BK], BF16, tag="pcs", bufs=1,
                                     name=f"pcs{B}_{slot}")
                    nc.vector.scalar_tensor_tensor(
                        out=pcs[:], in0=iota_sb[:],
                        scalar=sched_sb[:, p:p + 1], in1=pex[:],
                        op0=ALU.is_ge, op1=ALU.mult,
                        accum_out=partials[:, p:p + 1],
                    )
                    pm = pwork.tile([128, BK], BF16, tag="pm",
                                    name=f"pm{B}_{slot}")
                    nc.vector.tensor_mul(pm[:], pcs[:], mk[:])
                    if state["prev"] is not None:
                        av_stage(state["prev"], state["prev_pmt"])
                    state["prev"] = (pm, vtA, vtB, B, slot)

            # group 1: blocks 0 and 1 (need only KAG0 + VAG0) — emitted
            # BEFORE the KAG1/VAG1 triggers so their DRAM DMAs don't
            # conservatively wait on the later collectives.
            emit_block(0)
            emit_block(1)

            nc.gpsimd.collective_compute(
                "AllGather", ALU.bypass, replica_groups=RG,
                ins=[kq_in[1].opt()], outs=[kg[1].opt()],
            )
            nc.gpsimd.collective_compute(
                "AllGather", ALU.bypass, replica_groups=RG,
                ins=[vq_in[1].opt()], outs=[vg[1].opt()],
            )

            # group 2: blocks 2 and 3
            emit_block(2)
            emit_block(3)

            state["prev_pmt"] = tp_stage(state["prev"])
            av_stage(state["prev"], state["prev_pmt"])
            for slot in range(4):
                normalize_slot(slot)

    nc.compile()
    return nc


_NC_CACHE = None


def _get_nc():
    global _NC_CACHE
    if _NC_CACHE is None:
        _NC_CACHE = build()
    return _NC_CACHE


def make_in_maps(x, Wq, Wk, Wv, drop_mask):
    bf = ml_dtypes.bfloat16
    x = np.asarray(x, dtype=np.float32)
    Wq = np.asarray(Wq, dtype=np.float32)
    Wk = np.asarray(Wk, dtype=np.float32)
    Wv = np.asarray(Wv, dtype=np.float32)
    drop_mask = np.asarray(drop_mask, dtype=np.float32)

    xT = np.ascontiguousarray(x.T).astype(bf)           # [D, S]
    wqT = np.ascontiguousarray(Wq.T.astype(bf))         # [D, D]
    wvT = np.ascontiguousarray(Wv.T.astype(bf))         # [D, D]
    wkT = np.ascontiguousarray(Wk.T.astype(bf))         # [D, D]
    mask_bf = drop_mask.astype(bf)

    in_maps = []
    for c in range(NC):
        tl = owned_tiles(c)
        thr = np.array(
            [1024.0 * B - 128.0 * tl[slot]
             for slot in range(4) for B in range(KBMAX[slot])],
            dtype=np.float32,
        )
        in_maps.append({
            "xq": np.ascontiguousarray(
                np.concatenate([xT[:, 128 * t:128 * (t + 1)] for t in tl],
                               axis=1)),
            "wqT": wqT,
            "wvT": wvT,
            "wkT": wkT,
            "drop_mask": np.ascontiguousarray(
                np.concatenate(
                    [mask_bf[128 * t:128 * (t + 1)] for t in tl], axis=0)),
            "sched": np.ascontiguousarray(np.tile(thr[None, :], (128, 1))),
        })
    return in_maps


def assemble(results):
    full = np.zeros((S, D), dtype=np.float32)
    for c in range(NC):
        o = np.asarray(results[c]["out"], dtype=np.float32)
        for slot, t in enumerate(owned_tiles(c)):
            full[128 * t:128 * (t + 1)] = o[128 * slot:128 * (slot + 1)]
    return full


def kernel(x, Wq, Wk, Wv, drop_mask):
    nc = _get_nc()
    in_maps = make_in_maps(x, Wq, Wk, Wv, drop_mask)
    res = bass_utils.run_bass_kernel_spmd(nc, in_maps, core_ids=list(range(NC)))
    return assemble(res.results)


def kernel_profiled(x, Wq, Wk, Wv, drop_mask):
    """Like kernel(), but captures an NTFF profile; returns (out, exec_time_ns,
    trace_path)."""
    nc = _get_nc()
    in_maps = make_in_maps(x, Wq, Wk, Wv, drop_mask)
    res = bass_utils.run_bass_kernel_spmd(
        nc, in_maps, core_ids=list(range(NC)), trace=True)
    trace_path = None
    if res.instructions_and_trace is not None:
        trace_path = res.instructions_and_trace[1]
    return assemble(res.results), res.exec_time_ns, trace_path


# revision 12
# speedup vs baseline: 3.5991x; 3.5991x over previous
"""Distributed causal-attention-with-dropout kernel for 8 TRN2 NeuronCores, v17.

Architecture ("all-local projections", fully static SPMD graph):

- Host pre-formats inputs (layout only, all model FLOPs stay on device):
  each core receives xq = x^T columns of its 4 OWNED q-tiles
  {c, 15-c, 16+c, 31-c} (bf16), the FULL Wq^T / Wk^T / Wv^T (bf16,
  replicated), dropout-mask rows of its owned tiles (bf16), and the causal
  threshold table.  There is NO x gather at all.
- A tiny dummy AllGather with NO input dependency fires at t~0 so the
  collectives-runtime init barrier (~20+60us) overlaps the K projection.
- Tensor phase order: K proj -> V slots {0,1} -> V slots {2,3} -> Q proj
  -> attention.  Startup loads are interleaved (wk ki-chunk, xq ki-chunk)
  across sync+gpsimd queues and the K projection consumes ki in arrival
  order, so the PE starts at ~4us and stays dense (HAM stays un-throttled).
- K^T is AllGathered in k-tile-group chunks at FULL d_out depth:
  kq_in[g] = [2048, 256] = all d_out rows x the core's two owned q-tiles
  of group g (g=0: tiles {c,15-c} < 16; g=1: tiles {16+c,31-c}).  Score
  big-block B therefore needs ONLY kg[B//2].
- CC chain order = consumption order: KAG0, VAG0, KAG1, VAG1.  The
  KAG1/VAG1 triggers are EMITTED after the B=0/1 attention loads: DRAM
  DMAs conservatively wait on previously-emitted collectives, so B=0/1
  loads must precede the later triggers or they stall until KAG1 is done
  (measured 80us loss in v16).
- Attention: core c owns q-tiles {c, 15-c, 16+c, 31-c}; k-blocks are 1024
  wide, giving a ZERO-padding static schedule (slot s needs exactly s+1
  blocks; 10 pairs).  Causality enforced per-row by (iota(p-j) >= thr) * P
  on the vector engine; softmax without max-subtraction; denominators use
  pre-dropout sums.  Pair p's P-transposes and attn@V run after pair p+1's
  score matmuls (software pipeline).
"""

import math
import os
import sys
from contextlib import ExitStack

import numpy as np
import ml_dtypes

for _p in ("/opt/trn_rl_repo", "/root/.axon_site/_ro/trn_rl_repo"):
    if os.path.isdir(_p) and _p not in sys.path:
        sys.path.append(_p)

import concourse.bass as bass
import concourse.tile as tile
from concourse import bacc, mybir
from concourse import bass_utils
from concourse.masks import make_identity

S, D = 4096, 2048
NC = 8
SB = 512          # seq rows per core (4 owned 128-tiles)
BK = 1024         # big k-block width
NBIG = 4
KBMAX = (1, 2, 3, 4)
PBASE = (0, 1, 3, 6)
NPAIR = 10
SCALE = 1.0 / math.sqrt(float(D))
F32 = mybir.dt.float32
BF16 = mybir.dt.bfloat16
RG = [list(range(NC))]
ALU = mybir.AluOpType
AFT = mybir.ActivationFunctionType

# ki consumption order for the K projection: matches the 2-queue load
# arrival pattern (sync: wkA+xq ki 0..7, gpsimd: wkB+xq ki 8..15) so the
# first dp group issues matmuls as chunks land.
KI_ORDER = (0, 8, 1, 9, 2, 10, 3, 11, 4, 12, 5, 13, 6, 14, 7, 15)


def owned_tiles(c):
    return (c, 15 - c, 16 + c, 31 - c)


def tile_owner_slot(t):
    if t <= 7:
        return t, 0
    if t <= 15:
        return 15 - t, 1
    if t <= 23:
        return t - 16, 2
    return 31 - t, 3


# row of tile t inside its V AllGather chunk (chunk = t//16; within a
# chunk, rank blocks of 256 rows hold slots {0,1} or {2,3})
VROW2 = [256 * tile_owner_slot(t)[0] + 128 * (tile_owner_slot(t)[1] % 2)
         for t in range(32)]


def build():
    nc = bacc.Bacc("TRN2", target_bir_lowering=False, debug=False,
                   num_devices=NC)

    xq_in = nc.dram_tensor("xq", [D, SB], BF16, kind="ExternalInput").ap()
    wq_in = nc.dram_tensor("wqT", [D, D], BF16, kind="ExternalInput").ap()
    wv_in = nc.dram_tensor("wvT", [D, D], BF16, kind="ExternalInput").ap()
    wk_in = nc.dram_tensor("wkT", [D, D], BF16, kind="ExternalInput").ap()
    mask_in = nc.dram_tensor("drop_mask", [4 * 128, S], BF16,
                             kind="ExternalInput").ap()
    sched_in = nc.dram_tensor("sched", [128, NPAIR], F32,
                              kind="ExternalInput").ap()
    out_ext = nc.dram_tensor("out", [4 * 128, D], BF16,
                             kind="ExternalOutput").ap()

    with tile.TileContext(nc) as tc:
        with ExitStack() as es:
            dram = es.enter_context(tc.tile_pool(name="dram", bufs=1,
                                                 space="DRAM"))
            const = es.enter_context(tc.tile_pool(name="const", bufs=1))
            psum = es.enter_context(tc.tile_pool(name="psum", bufs=1,
                                                 space="PSUM"))

            # ---------------- DRAM scratch ----------------
            dummy_in = dram.tile([1, NPAIR], F32, name="dummy_in")
            dummy_out = dram.tile([NC, NPAIR], F32, addr_space="Shared",
                                  name="dummy_out")
            # V contributions split by slot-pair: chunk 0 = slots {0,1}
            # (true tiles 0..15), chunk 1 = slots {2,3} (tiles 16..31).
            vq_in = [dram.tile([256, D], BF16, name=f"vq_in{h}")
                     for h in range(2)]
            vg = [dram.tile([NC * 256, D], BF16, addr_space="Shared",
                            name=f"vg{h}") for h in range(2)]
            # per-core K^T contribution in k-tile-group chunks at FULL
            # d_out depth: chunk g = [2048 d_out rows, 256 cols] covering
            # the core's two owned q-tiles of group g.
            kq_in = [dram.tile([D, 256], BF16, name=f"kq_in{g}")
                     for g in range(2)]
            kg = [dram.tile([NC * D, 256], BF16, addr_space="Shared",
                            name=f"kg{g}") for g in range(2)]

            # dummy AllGather first, with NO input dependency (dummy_in is
            # never written): the CC trigger fires at t~0 so the
            # collectives-init barrier overlaps the K projection.
            nc.gpsimd.collective_compute(
                "AllGather", ALU.bypass, replica_groups=RG,
                ins=[dummy_in.opt()], outs=[dummy_out.opt()],
            )

            # ---------------- weight / activation loads ----------------
            sched_sb = const.tile([128, NPAIR], F32, name="sched_sb")
            nc.sync.dma_start(sched_sb[:], sched_in)

            att = es.enter_context(tc.tile_pool(name="att", bufs=1))
            qt_sb = att.tile([128, 16, SB], BF16, name="qt_sb")

            qes = ExitStack()
            xqp = qes.enter_context(tc.tile_pool(name="xqp", bufs=1))
            xq_sb = xqp.tile([128, 16, SB], BF16, name="xq_sb")
            stagep = qes.enter_context(tc.tile_pool(name="stagep", bufs=2))
            # Weight pool: 4 buffers of [128, 8, D] (one ki-half each).
            # Rotation: wkA, wkB, wvA, wvB live first; wqA/wqB rotate into
            # wkA/wkB's buffers once the K projection finishes.
            wpool = qes.enter_context(tc.tile_pool(name="wpool", bufs=4))

            def wh_tile(name):
                return wpool.tile([128, 8, D], BF16, tag="wh", name=name)

            wkA = wh_tile("wkA"); wkB = wh_tile("wkB")
            wvA = wh_tile("wvA"); wvB = wh_tile("wvB")

            # fine-grained interleaved startup loads:
            # sync:   (wkA ki, xq ki) for ki 0..7
            # gpsimd: (wkB ki, xq ki) for ki 8..15
            # scalar: wvA, wvB (then wq, masks later)
            for ki in range(8):
                nc.sync.dma_start(wkA[:, ki, :],
                                  wk_in[128 * ki:128 * (ki + 1), :])
                nc.sync.dma_start(xq_sb[:, ki, :],
                                  xq_in[128 * ki:128 * (ki + 1), :])
            for ki in range(8, 16):
                nc.gpsimd.dma_start(wkB[:, ki - 8, :],
                                    wk_in[128 * ki:128 * (ki + 1), :])
                nc.gpsimd.dma_start(xq_sb[:, ki, :],
                                    xq_in[128 * ki:128 * (ki + 1), :])
            nc.scalar.dma_start(
                wvA[:], wv_in[0:1024, :].rearrange("(k p) d -> p k d", p=128))
            nc.scalar.dma_start(
                wvB[:], wv_in[1024:2048, :].rearrange("(k p) d -> p k d",
                                                      p=128))

            # ---------------- constants (gpsimd engine, after triggers) ---
            ident_sb = const.tile([128, 128], BF16, name="ident_sb")
            make_identity(nc, ident_sb[:])
            iota_sb = const.tile([128, BK], F32, name="iota_sb")
            nc.gpsimd.iota(
                iota_sb[:], pattern=[[-1, BK]], base=0, channel_multiplier=1,
                allow_small_or_imprecise_dtypes=True,
            )

            partials = const.tile([128, NPAIR], F32, name="partials")
            den = const.tile([128, 4], F32, name="den")
            rec = const.tile([128, 4], F32, name="rec")

            # ------- phase K: local K^T (all d_out, own q), k-tile-group
            # chunked output at full depth -------
            for dp in range(8):
                psA = psum.tile([128, BK], F32, tag="pw", bufs=3,
                                name=f"pskA{dp}")
                psB = psum.tile([128, BK], F32, tag="pw", bufs=3,
                                name=f"pskB{dp}")
                for i, ki in enumerate(KI_ORDER):
                    wkh = wkA if ki < 8 else wkB
                    nc.tensor.matmul(
                        psA[:, 0:SB], lhsT=wkh[:, ki % 8, 256 * dp:
                                               256 * dp + 128],
                        rhs=xq_sb[:, ki, :],
                        start=(i == 0), stop=(i == 15),
                    )
                    nc.tensor.matmul(
                        psB[:, 0:SB], lhsT=wkh[:, ki % 8, 256 * dp + 128:
                                               256 * dp + 256],
                        rhs=xq_sb[:, ki, :],
                        start=(i == 0), stop=(i == 15),
                    )
                for half, ps in ((0, psA), (1, psB)):
                    dt = 2 * dp + half
                    kst = stagep.tile([128, SB], BF16, tag="kst", bufs=4,
                                      name=f"kst{dt}")
                    nc.vector.tensor_copy(kst[:], ps[:, 0:SB])
                    # group 0 = slots {0,1} = cols 0:256; group 1 = 256:512
                    nc.sync.dma_start(
                        kq_in[0][128 * dt:128 * (dt + 1), :], kst[:, 0:256])
                    nc.sync.dma_start(
                        kq_in[1][128 * dt:128 * (dt + 1), :], kst[:, 256:512])

            nc.gpsimd.collective_compute(
                "AllGather", ALU.bypass, replica_groups=RG,
                ins=[kq_in[0].opt()], outs=[kg[0].opt()],
            )

            # ------- phase V: natural layout; slots {0,1} then {2,3} ------
            def v_slot(st):
                vq_dst = vq_in[st // 2].rearrange("(t p) d -> p t d", p=128)
                for h in range(2):
                    ps = psum.tile([128, BK], F32, tag="pw", bufs=3,
                                   name=f"psv{st}_{h}")
                    for ki in range(16):
                        for n2 in range(2):
                            nc.tensor.matmul(
                                ps[:, 512 * n2:512 * (n2 + 1)],
                                lhsT=xq_sb[:, ki, 128 * st:128 * (st + 1)],
                                rhs=(wvA if ki < 8 else wvB)[
                                    :, ki % 8, BK * h + 512 * n2:
                                    BK * h + 512 * (n2 + 1)],
                                start=(ki == 0), stop=(ki == 15),
                                skip_group_check=True,
                            )
                    vst = stagep.tile([128, BK], BF16, tag="vst", bufs=3,
                                      name=f"vst{st}_{h}")
                    nc.vector.tensor_copy(vst[:], ps[:])
                    nc.sync.dma_start(
                        vq_dst[:, st % 2, BK * h:BK * (h + 1)], vst[:])

            v_slot(0)
            v_slot(1)
            nc.gpsimd.collective_compute(
                "AllGather", ALU.bypass, replica_groups=RG,
                ins=[vq_in[0].opt()], outs=[vg[0].opt()],
            )
            v_slot(2)
            v_slot(3)

            # Wq^T halves rotate into Wk^T's buffers (dep: K matmuls done).
            wqA = wh_tile("wqA"); wqB = wh_tile("wqB")
            nc.scalar.dma_start(
                wqA[:], wq_in[0:1024, :].rearrange("(k p) d -> p k d", p=128))
            nc.scalar.dma_start(
                wqB[:], wq_in[1024:2048, :].rearrange("(k p) d -> p k d",
                                                      p=128))

            # ------- phase Q: local Q^T projection ------
            for dp in range(8):
                psA = psum.tile([128, BK], F32, tag="pw", bufs=3,
                                name=f"psqA{dp}")
                psB = psum.tile([128, BK], F32, tag="pw", bufs=3,
                                name=f"psqB{dp}")
                for ki in range(16):
                    wqh = wqA if ki < 8 else wqB
                    nc.tensor.matmul(
                        psA[:, 0:SB], lhsT=wqh[:, ki % 8, 256 * dp:
                                               256 * dp + 128],
                        rhs=xq_sb[:, ki, :],
                        start=(ki == 0), stop=(ki == 15),
                    )
                    nc.tensor.matmul(
                        psB[:, 0:SB], lhsT=wqh[:, ki % 8, 256 * dp + 128:
                                               256 * dp + 256],
                        rhs=xq_sb[:, ki, :],
                        start=(ki == 0), stop=(ki == 15),
                    )
                nc.vector.tensor_copy(qt_sb[:, 2 * dp, :], psA[:, 0:SB])
                nc.vector.tensor_copy(qt_sb[:, 2 * dp + 1, :],
                                      psB[:, 0:SB])
            qes.close()

            # ---------------- attention (software-pipelined) ----------------
            accp = es.enter_context(tc.tile_pool(name="accp", bufs=1))
            ktl = es.enter_context(tc.tile_pool(name="ktl", bufs=3))
            vtl = es.enter_context(tc.tile_pool(name="vtl", bufs=3))
            mkl = es.enter_context(tc.tile_pool(name="mkl", bufs=4))
            pwork = es.enter_context(tc.tile_pool(name="pwork", bufs=2))

            acc = [accp.tile([128, D], F32, name=f"acc{t}") for t in range(4)]

            def normalize_slot(slot):
                obf = pwork.tile([128, D], BF16, tag="obf", bufs=2,
                                 name=f"obf{slot}")
                nc.vector.tensor_reduce(
                    den[:, slot:slot + 1],
                    partials[:, PBASE[slot]:PBASE[slot] + KBMAX[slot]],
                    axis=mybir.AxisListType.X, op=ALU.add,
                )
                nc.vector.reciprocal(rec[:, slot:slot + 1],
                                     den[:, slot:slot + 1])
                nc.vector.tensor_scalar_mul(
                    obf[:], acc[slot][:], rec[:, slot:slot + 1])
                nc.scalar.dma_start(
                    out_ext[128 * slot:128 * (slot + 1), :], obf[:])

            def tp_stage(st):
                pm, vtA, vtB, B, slot = st
                pmt = pwork.tile([128, 8, 128], BF16, tag="pmt",
                                 name=f"pmt{B}_{slot}")
                for j in range(8):
                    tp = psum.tile([128, 128], BF16, tag="tp", bufs=2,
                                   name=f"tp{B}_{slot}_{j}")
                    nc.tensor.matmul(
                        tp[:], lhsT=pm[:, 128 * j:128 * (j + 1)],
                        rhs=ident_sb[:], is_transpose=True,
                        skip_group_check=True)
                    nc.scalar.copy(pmt[:, j, :], tp[:])
                return pmt

            def av_stage(st, pmt):
                pm, vtA, vtB, B, slot = st
                for h, vt in ((0, vtA), (1, vtB)):
                    av = psum.tile([128, BK], F32, tag="pw", bufs=3,
                                   name=f"av{B}_{slot}_{h}")
                    for j in range(8):
                        for n2 in range(2):
                            nc.tensor.matmul(
                                av[:, 512 * n2:512 * (n2 + 1)],
                                lhsT=pmt[:, j, :],
                                rhs=vt[:, j, 512 * n2:512 * (n2 + 1)],
                                start=(j == 0), stop=(j == 7),
                                skip_group_check=True,
                            )
                    if B == 0:
                        nc.vector.tensor_copy(
                            acc[slot][:, BK * h:BK * (h + 1)], av[:])
                    else:
                        nc.vector.scalar_tensor_tensor(
                            out=acc[slot][:, BK * h:BK * (h + 1)],
                            in0=av[:], scalar=1.0,
                            in1=acc[slot][:, BK * h:BK * (h + 1)],
                            op0=ALU.mult, op1=ALU.add,
                        )

            state = {"prev": None, "prev_pmt": None}

            def emit_block(B):
                # kt halves: ktA = d_out rows 0..1023 (score ki 0..7),
                # ktB = rows 1024..2047 (ki 8..15), both from kg[B//2].
                # All 8 tiles of big-block B share the same slot parity:
                # columns 128*(B%2) of the AG chunk.
                ktA = ktl.tile([128, 8, BK], BF16, tag="kt", name=f"ktA{B}")
                ktB = ktl.tile([128, 8, BK], BF16, tag="kt", name=f"ktB{B}")
                g = B // 2
                coff = 128 * (B % 2)
                # NOTE: never put CC-completion-dependent DMAs on gpsimd —
                # they would block the later CC triggers queued behind them
                # (measured: collectives-init barrier ballooned to 1.4ms).
                for H, kth, eng in ((0, ktA, nc.sync), (1, ktB, nc.scalar)):
                    for j in range(8):
                        t = 8 * B + j
                        c, _s = tile_owner_slot(t)
                        eng.dma_start(
                            kth[:, :, 128 * j:128 * (j + 1)],
                            kg[g][D * c + BK * H:D * c + BK * (H + 1),
                                  coff:coff + 128]
                            .rearrange("(k p) q -> p k q", p=128),
                        )
                vtA = vtl.tile([128, 8, BK], BF16, tag="vt", name=f"vtA{B}")
                vtB = vtl.tile([128, 8, BK], BF16, tag="vt", name=f"vtB{B}")
                for j in range(8):
                    t = 8 * B + j
                    vgx = vg[t // 16]
                    r0 = VROW2[t]
                    nc.sync.dma_start(vtA[:, j, :], vgx[r0:r0 + 128, 0:BK])
                    nc.scalar.dma_start(vtB[:, j, :], vgx[r0:r0 + 128, BK:D])
                for slot in range(B, 4):
                    p = PBASE[slot] + B
                    mk = mkl.tile([128, BK], BF16, tag="mk",
                                  name=f"mk{B}_{slot}")
                    nc.scalar.dma_start(
                        mk[:],
                        mask_in[128 * slot:128 * (slot + 1),
                                BK * B:BK * (B + 1)],
                    )
                    sc = psum.tile([128, BK], F32, tag="pw", bufs=3,
                                   name=f"sc{B}_{slot}")
                    for ki in range(16):
                        if ki == 8 and state["prev"] is not None:
                            # interleave prev pair's P-transposes here so
                            # the pmt copies finish before its attn@V
                            state["prev_pmt"] = tp_stage(state["prev"])
                        kth = ktA if ki < 8 else ktB
                        for n2 in range(2):
                            nc.tensor.matmul(
                                sc[:, 512 * n2:512 * (n2 + 1)],
                                lhsT=qt_sb[:, ki, 128 * slot:128 * (slot + 1)],
                                rhs=kth[:, ki % 8, 512 * n2:512 * (n2 + 1)],
                                start=(ki == 0), stop=(ki == 15),
                                skip_group_check=True,
                            )
                    pex = pwork.tile([128, BK], BF16, tag="pex", bufs=1,
                                     name=f"pex{B}_{slot}")
                    nc.scalar.activation(pex[:], sc[:], AFT.Exp, scale=SCALE)
                    pcs = pwork.tile([128, BK], BF16, tag="pcs", bufs=1,
                                     name=f"pcs{B}_{slot}")
                    nc.vector.scalar_tensor_tensor(
                        out=pcs[:], in0=iota_sb[:],
                        scalar=sched_sb[:, p:p + 1], in1=pex[:],
                        op0=ALU.is_ge, op1=ALU.mult,
                        accum_out=partials[:, p:p + 1],
                    )
                    pm = pwork.tile([128, BK], BF16, tag="pm",
                                    name=f"pm{B}_{slot}")
                    nc.vector.tensor_mul(pm[:], pcs[:], mk[:])
                    if state["prev"] is not None:
                        av_stage(state["prev"], state["prev_pmt"])
                    state["prev"] = (pm, vtA, vtB, B, slot)

            # group 1: blocks 0 and 1 (need only KAG0 + VAG0) — emitted
            # BEFORE the KAG1/VAG1 triggers so their DRAM DMAs don't
            # conservatively wait on the later collectives.
            emit_block(0)
            emit_block(1)

            nc.gpsimd.collective_compute(
                "AllGather", ALU.bypass, replica_groups=RG,
                ins=[kq_in[1].opt()], outs=[kg[1].opt()],
            )
            nc.gpsimd.collective_compute(
                "AllGather", ALU.bypass, replica_groups=RG,
                ins=[vq_in[1].opt()], outs=[vg[1].opt()],
            )

            # group 2: blocks 2 and 3
            emit_block(2)
            emit_block(3)

            state["prev_pmt"] = tp_stage(state["prev"])
            av_stage(state["prev"], state["prev_pmt"])
            for slot in range(4):
                normalize_slot(slot)

    nc.compile()
    return nc


_NC_CACHE = None


def _get_nc():
    global _NC_CACHE
    if _NC_CACHE is None:
        _NC_CACHE = build()
    return _NC_CACHE


def make_in_maps(x, Wq, Wk, Wv, drop_mask):
    bf = ml_dtypes.bfloat16
    x = np.asarray(x, dtype=np.float32)
    Wq = np.asarray(Wq, dtype=np.float32)
    Wk = np.asarray(Wk, dtype=np.float32)
    Wv = np.asarray(Wv, dtype=np.float32)
    drop_mask = np.asarray(drop_mask, dtype=np.float32)

    xT = np.ascontiguousarray(x.T).astype(bf)           # [D, S]
    wqT = np.ascontiguousarray(Wq.T.astype(bf))         # [D, D]
    wvT = np.ascontiguousarray(Wv.T.astype(bf))         # [D, D]
    wkT = np.ascontiguousarray(Wk.T.astype(bf))         # [D, D]
    mask_bf = drop_mask.astype(bf)

    in_maps = []
    for c in range(NC):
        tl = owned_tiles(c)
        thr = np.array(
            [1024.0 * B - 128.0 * tl[slot]
             for slot in range(4) for B in range(KBMAX[slot])],
            dtype=np.float32,
        )
        in_maps.append({
            "xq": np.ascontiguousarray(
                np.concatenate([xT[:, 128 * t:128 * (t + 1)] for t in tl],
                               axis=1)),
            "wqT": wqT,
            "wvT": wvT,
            "wkT": wkT,
            "drop_mask": np.ascontiguousarray(
                np.concatenate(
                    [mask_bf[128 * t:128 * (t + 1)] for t in tl], axis=0)),
            "sched": np.ascontiguousarray(np.tile(thr[None, :], (128, 1))),
        })
    return in_maps


def assemble(results):
    full = np.zeros((S, D), dtype=np.float32)
    for c in range(NC):
        o = np.asarray(results[c]["out"], dtype=np.float32)
        for slot, t in enumerate(owned_tiles(c)):
            full[128 * t:128 * (t + 1)] = o[128 * slot:128 * (slot + 1)]
    return full


def kernel(x, Wq, Wk, Wv, drop_mask):
    nc = _get_nc()
    in_maps = make_in_maps(x, Wq, Wk, Wv, drop_mask)
    res = bass_utils.run_bass_kernel_spmd(nc, in_maps, core_ids=list(range(NC)))
    return assemble(res.results)


def kernel_profiled(x, Wq, Wk, Wv, drop_mask):
    """Like kernel(), but captures an NTFF profile; returns (out, exec_time_ns,
    trace_path)."""
    nc = _get_nc()
    in_maps = make_in_maps(x, Wq, Wk, Wv, drop_mask)
    res = bass_utils.run_bass_kernel_spmd(
        nc, in_maps, core_ids=list(range(NC)), trace=True)
    trace_path = None
    if res.instructions_and_trace is not None:
        trace_path = res.instructions_and_trace[1]
    return assemble(res.results), res.exec_time_ns, trace_path
